# revision 58
# baseline (speedup 1.0000x reference)
"""Self-contained Trainium2 Bass kernel for a 2-layer GCN + FC + log_softmax.

Distribution: nodes sharded across 8 NeuronCores (12500 rows each); edges
partitioned by destination node so each core's scatter-add is local; gather
tables replicated/exchanged; small weights replicated.

Key structure (per core):
  - Layer 1 aggregates in x-space (A·(dinv*x) then @W1 at flush time, since
    aggregation and the dense matmul commute) so its gather table is a host
    input — no stage-A compute or startup AllGather at all.
  - Aggregation: per 125-dst-node tile, PSUM = ident @ x_own[tile] (self
    loops) + sum over 128-edge chunks onehot.T @ table[src]. Source rows come
    from four 25000-row banks via dma_gather (fp8 rows padded to 256 B — the
    SWDGE descriptor minimum), one bank per SWDGE queue so all four GpSimd
    Q7 pairs generate descriptors concurrently (descriptor generation is the
    critical resource, ~9 ns/row/queue). One-hot matrices are precomputed on
    the host in fp8 (128-wide for fast weight load) and streamed in via HWDGE
    so no compute engine touches them — 2-port DVE work would lock GpSimd
    out of its SBUF descriptor rings.
  - Flush: h = relu(dinv·(acc@W)) (+ rank-1 sqdeg×b if biases are nonzero);
    layer-1 flush emits the fp8 layer-2 table rows (dinv·h1), AllGathered in
    a 3+1 bank split so the big collective hides inside layer 1 and only a
    small one remains at the layer boundary; layer-2 flush applies W2, Wfc
    and a fused log_softmax.
"""
import math

import numpy as np
import ml_dtypes

import concourse.bass as bass
import concourse.mybir as mybir
import concourse.tile as tile
from concourse import bacc, bass_utils

BF16 = ml_dtypes.bfloat16

# Problem contract (hardcoded; must match setup_inputs()).
N_NODES = 100000
N_EDGES = 1600000
D = 128
DOUT = 40

N_CORES = 8
TILE_N = 125            # dst nodes per PSUM tile
BANK = 25000            # gather table bank rows (4 equal banks -> 4 queues)
TB = 5                  # dst tiles per gather batch
CHUNK = 128             # edges per matmul chunk
MAXC = 24               # max chunks per dma_gather call
KM = 24                 # max chunks per one-hot build / matmul run
FP32 = mybir.dt.float32
BF16_T = mybir.dt.bfloat16
I16 = mybir.dt.int16
FP16 = mybir.dt.float16
FP8 = mybir.dt.float8e4
F8NP = mybir.dt.np(mybir.dt.float8e4)
DW = 256                # fp8 table row width (128 data + 128 pad = 256B)


# ---------------------------------------------------------------------------
# Host preprocessing
# ---------------------------------------------------------------------------

def _preprocess(x, edge_index):
    n = x.shape[0]
    slice_n = n // N_CORES
    n_tiles = slice_n // TILE_N
    n_banks = (n + BANK - 1) // BANK

    ei = np.asarray(edge_index, np.int64)
    src = ei[0]
    dst = ei[1]
    # degree includes the self-loops the reference adds
    deg = (np.bincount(dst, minlength=n) + 1).astype(np.float32)
    dinv = 1.0 / np.sqrt(deg)
    sqdeg = np.sqrt(deg)

    # Table rows live in the "split AllGather" layout: part 0 concatenates the
    # cores' first 9375 rows (banks 0-2), part 1 the last 3125 (bank 3) — so
    # banks 0-2 are gatherable after the big mid-layer AllGather and only the
    # small part-1 AllGather remains at the layer boundary.
    P0 = (slice_n // 4) * 3
    P1 = slice_n // 4
    u_ = np.arange(n, dtype=np.int64)
    r_, i_ = u_ // slice_n, u_ % slice_n
    perm_node = np.where(i_ < P0, r_ * P0 + i_,
                         N_CORES * P0 + r_ * P1 + (i_ - P0))
    perm = perm_node[src]

    tile_id = dst // TILE_N                       # global tile (core*n_tiles+t)
    batch_g = (tile_id % n_tiles) // TB           # batch within core
    core_id = tile_id // n_tiles
    bank_id = perm // BANK
    order = np.lexsort((tile_id, bank_id, batch_g, core_id))
    src_s = perm[order]

    batches = [list(range(b, min(b + TB, n_tiles))) for b in range(0, n_tiles, TB)]
    n_batches = len(batches)
    SENT = TB * TILE_N                            # seg sentinel (no tile matches)

    # counts per (core, tile, bank) and per (core, batch, bank)
    key_ctb = (tile_id * n_banks + bank_id)
    cnt_tb = np.bincount(key_ctb, minlength=N_CORES * n_tiles * n_banks).reshape(
        N_CORES, n_tiles, n_banks
    )
    cnt_bb = np.zeros((N_CORES, n_batches, n_banks), np.int64)
    for bi, tiles in enumerate(batches):
        cnt_bb[:, bi, :] = cnt_tb[:, tiles, :].sum(axis=1)
    nbk_bb = ((cnt_bb + CHUNK - 1) // CHUNK).max(axis=0)      # [n_batches, n_banks]

    # group start offsets in the sorted edge array, keyed by (core,batch,bank,tile)
    # sorted order is (core, batch, bank, tile) by the lexsort above
    starts = {}
    pos = 0
    for c in range(N_CORES):
        for bi, tiles in enumerate(batches):
            for k in range(n_banks):
                for t in tiles:
                    m = int(cnt_tb[c, t, k])
                    starts[(c, bi, k, t)] = (pos, m)
                    pos += m
    assert pos == len(src_s)

    # static chunk structure: per batch, per bank, chunk cols; per-tile chunk
    # membership = union over cores
    batch_meta = []
    col = 0
    for bi, tiles in enumerate(batches):
        bm = {"banks": [], "tiles": {t: [] for t in tiles}}
        for k in range(n_banks):
            nbk = int(nbk_bb[bi, k])
            c0 = col
            # per-core tile spans within this (batch, bank) stream
            present = {t: set() for t in tiles}
            for c in range(N_CORES):
                off = 0
                for t in tiles:
                    m = starts[(c, bi, k, t)][1]
                    if m > 0:
                        j0, j1 = off // CHUNK, (off + m - 1) // CHUNK
                        for j in range(j0, j1 + 1):
                            present[t].add(c0 + j)
                    off += m
            for t in tiles:
                for cc in sorted(present[t]):
                    bm["tiles"][t].append((k, cc))
            bm["banks"].append((k, c0, nbk))
            col += nbk
        # order each tile's chunk list by (bank, col) -- already appended that way
        batch_meta.append(bm)
    total_chunks = col

    # Per-core edge arrays in global chunk-column order.
    per_core = []
    for c in range(N_CORES):
        idx_mat = np.zeros((total_chunks, CHUNK), np.int16)
        seg_mat = np.full((total_chunks, CHUNK), SENT, np.int16)
        for bi, tiles in enumerate(batches):
            bm = batch_meta[bi]
            node0 = tiles[0] * TILE_N            # batch-local dst base (per-core)
            for k, c0, nbk in bm["banks"]:
                parts_i, parts_s = [], []
                for t in tiles:
                    s0, m = starts[(c, bi, k, t)]
                    parts_i.append(src_s[s0 : s0 + m] - k * BANK)
                    parts_s.append(dst[order[s0 : s0 + m]] - c * slice_n - node0)
                eidx = np.concatenate(parts_i).astype(np.int16)
                es = np.concatenate(parts_s).astype(np.int16)
                m = len(eidx)
                pad = nbk * CHUNK - m
                eidx = np.concatenate([eidx, np.zeros(pad, np.int16)])
                es = np.concatenate([es, np.full(pad, SENT, np.int16)])
                idx_mat[c0 : c0 + nbk] = eidx.reshape(nbk, CHUNK)
                seg_mat[c0 : c0 + nbk] = es.reshape(nbk, CHUNK)
        per_core.append((idx_mat, seg_mat))

    # one-hot instance table: per (batch, tile), one instance per chunk in the
    # tile's chunk list, contiguous in (batch, tile, chunk) order
    inst_start = {}
    inst_cols = []
    inst_off = []
    for bi, tiles in enumerate(batches):
        bm = batch_meta[bi]
        for tl, t in enumerate(tiles):
            inst_start[(bi, t)] = len(inst_cols)
            for k, cc in bm["tiles"][t]:
                inst_cols.append(cc)
                inst_off.append(tl * TILE_N)
    maxi = max(len(batch_meta[bi]["tiles"][t])
               for bi, tiles in enumerate(batches) for t in tiles)

    meta = {
        "n": n, "slice_n": slice_n, "n_tiles": n_tiles, "n_banks": n_banks,
        "batches": batches, "batch_meta": batch_meta, "total_chunks": total_chunks,
        "perm": perm_node,
        "inst_start": inst_start, "inst_cols": np.array(inst_cols),
        "inst_off": np.array(inst_off), "n_inst": len(inst_cols), "maxi": maxi,
    }
    return per_core, meta, dinv, sqdeg


def _pack_idx(idx_mat, meta):
    """Wrap chunk-major indices into the dma_gather [16, n/16] layout per
    (batch, bank) gather call, concatenated along the free dim, replicated to
    128 partitions. Returns ([128, S_total] int16, per-batch [start, len])."""
    spans = []
    blocks = []
    s = 0
    for bm in meta["batch_meta"]:
        s0 = s
        for k, c0, nbk in bm["banks"]:
            if nbk == 0:
                continue
            for g0 in range(0, nbk, MAXC):
                gl = min(MAXC, nbk - g0)
                flat = idx_mat[c0 + g0 : c0 + g0 + gl].reshape(-1)   # [gl*128]
                wrapped = flat.reshape(-1, 16).T                     # [16, gl*8]
                blocks.append(wrapped)
                s += wrapped.shape[1]
        spans.append((s0, s - s0))
    packed = np.concatenate(blocks, axis=1) if blocks else np.zeros((16, 0), np.int16)
    packed = np.tile(packed, (8, 1)).copy()                    # [128, S]
    return packed, spans


def _pack_dinv(v, slice0, meta, dtype):
    """[128, n_tiles]: partition p, col t = v[slice0 + t*TILE_N + p] (p<125)."""
    n_tiles = meta["n_tiles"]
    out = np.zeros((128, n_tiles), dtype)
    sl = v[slice0 : slice0 + n_tiles * TILE_N].reshape(n_tiles, TILE_N)
    out[:TILE_N, :] = sl.T
    return out


# ---------------------------------------------------------------------------
# Device kernel builder
# ---------------------------------------------------------------------------

def _build(meta, s_total, has_bias):
    n = meta["n"]
    slice_n = meta["slice_n"]
    n_tiles = meta["n_tiles"]
    n_banks = meta["n_banks"]
    total_chunks = meta["total_chunks"]
    n_a_tiles = math.ceil(slice_n / 128)

    nc = bacc.Bacc("TRN2", target_bir_lowering=False, debug=False,
                   num_devices=N_CORES, num_swdge_queues=4,
                   dynamic_dma_scratch_size=32768)

    # inputs
    xgt = [None, None]   # full dinv*x gather table, permuted, split 3+1
    xgo = [None, None]   # this core's own dinv*x rows (for self-loops)
    w1 = nc.dram_tensor("w1", [D, D], BF16_T, kind="ExternalInput")
    w2 = nc.dram_tensor("w2", [D, D], BF16_T, kind="ExternalInput")
    wfc = nc.dram_tensor("wfc", [D, DOUT], BF16_T, kind="ExternalInput")
    brows = nc.dram_tensor("brows", [4, D], BF16_T, kind="ExternalInput")
    # brows rows: 0=b1, 1=b2, 2=bfc (padded), 3=ones
    dinvp = nc.dram_tensor("dinvp", [128, n_tiles], FP32, kind="ExternalInput")
    sqdegp = (nc.dram_tensor("sqdegp", [1, slice_n], BF16_T, kind="ExternalInput")
              if has_bias else None)
    idx1 = nc.dram_tensor("idx1", [128, s_total], I16, kind="ExternalInput")
    mhot = nc.dram_tensor("mhot", [128, meta["n_inst"] * 128], FP8,
                          kind="ExternalInput")
    maxi = meta["maxi"]

    out = nc.dram_tensor("out", [slice_n, DOUT], FP32, kind="ExternalOutput")

    # internal dram — tables and bounces split 3+1: part 0 = banks 0-2
    # (9375 rows/core), part 1 = bank 3 (3125 rows/core)
    PR = [(slice_n // 4) * 3, slice_n // 4]
    NP_ = [N_CORES * PR[0], N_CORES * PR[1]]
    for q in range(2):
        xgt[q] = nc.dram_tensor(f"xgt{q}", [NP_[q], DW], FP8, kind="ExternalInput")
        xgo[q] = nc.dram_tensor(f"xgo{q}", [PR[q], DW], FP8, kind="ExternalInput")
    g2_bnc = [nc.dram_tensor(f"g2_bounce{q}", [PR[q], DW], FP8) for q in range(2)]
    g2_tab = [nc.dram_tensor(f"g2_table{q}", [NP_[q], DW], FP8, addr_space="Shared")
              for q in range(2)]

    with tile.TileContext(nc) as tc:
        with (
            tc.tile_pool(name="const", bufs=1) as constp,
            tc.tile_pool(name="msg", bufs=16) as msgp,
            tc.tile_pool(name="idxp", bufs=6) as idxp,
            tc.tile_pool(name="glp", bufs=4) as glp,
            tc.tile_pool(name="mp", bufs=8) as mp,
            tc.tile_pool(name="fl", bufs=4) as flp,
            tc.tile_pool(name="acc", bufs=3, space="PSUM") as accp,
            tc.tile_pool(name="tps", bufs=2, space="PSUM") as tpsp,
            tc.tile_pool(name="gps", bufs=2, space="PSUM") as gpsp,
        ):
            # constants
            w1_t = constp.tile([D, D], BF16_T, tag="w1")
            nc.sync.dma_start(out=w1_t[:], in_=w1[:, :])
            w2_t = constp.tile([D, D], BF16_T, tag="w2")
            nc.sync.dma_start(out=w2_t[:], in_=w2[:, :])
            wfc_t = constp.tile([D, DOUT], BF16_T, tag="wfc")
            nc.sync.dma_start(out=wfc_t[:], in_=wfc[:, :])
            brow_ts = []
            for r in range(4):
                bt = constp.tile([1, D], BF16_T, tag=f"brow{r}")
                nc.sync.dma_start(out=bt[:], in_=brows[r : r + 1, :])
                brow_ts.append(bt)
            dinv_t = constp.tile([128, n_tiles], FP32, tag="dinvp")
            nc.sync.dma_start(out=dinv_t[:], in_=dinvp[:, :])
            if has_bias:
                sqdeg_t = constp.tile([1, slice_n], BF16_T, tag="sqdegp")
                nc.sync.dma_start(out=sqdeg_t[:], in_=sqdegp[:, :])
            ident_t = constp.tile([128, 128], BF16_T, tag="ident")
            from concourse.masks import make_identity
            make_identity(nc, ident_t[:])
            ident8_t = constp.tile([128, 128], FP8, tag="ident8")
            make_identity(nc, ident8_t[:])

            # ---------------- Stage A ----------------
            # one bulk input load per half; matmuls slice it; grouped stores
            def ag(bounce_h, table_h):
                nc.gpsimd.collective_compute(
                    "AllGather", mybir.AluOpType.bypass,
                    ins=[bounce_h[:, :]], outs=[table_h[:, :]],
                    replica_groups=[list(range(N_CORES))],
                )

            # ---------------- aggregation layers ----------------
            _regs = {}

            def nidx_reg(v):
                if v not in _regs:
                    _regs[v] = nc.gpsimd.to_reg(v)
                return _regs[v]

            n_batches = len(meta["batches"])
            per_q = n_batches // 4

            def issue_batch(idx_dram, tables, bounces, spans, bi, tiles):
                bm = meta["batch_meta"][bi]
                s0, slen = spans[bi]
                it = idxp.tile([128, slen], I16, tag="idx")
                nc.sync.dma_start(out=it[:], in_=idx_dram[:, s0 : s0 + slen])

                # host-precomputed one-hot tiles, one per dst tile
                mts = []
                for ti, t in enumerate(tiles):
                    i0 = meta["inst_start"][(bi, t)]
                    ni = len(bm["tiles"][t])
                    m_t = mp.tile([128, maxi, 128], FP8, tag="m")
                    eng = nc.sync if ti % 2 == 0 else nc.scalar
                    eng.dma_start(
                        out=m_t[:, :ni, :],
                        in_=mhot[:, i0 * 128 : (i0 + ni) * 128]
                            .rearrange("p (i f) -> p i f", f=128),
                    )
                    mts.append(m_t)

                # local g rows for the batch's self-loop matmuls:
                # glt[p, j, :] = bounce_half[row0 + j*TILE_N + p, :]
                ntl = len(tiles)
                q = 0 if (tiles[-1] + 1) * TILE_N <= PR[0] else 1
                row0 = tiles[0] * TILE_N - (0 if q == 0 else PR[0])
                glt = glp.tile([128, ntl, DW], FP8, tag="gl")
                nc.scalar.dma_start(
                    out=glt[:TILE_N, :, :],
                    in_=bounces[q][row0 : row0 + ntl * TILE_N, :]
                        .rearrange("(t p) d -> p t d", p=TILE_N),
                )

                # bank gathers, split to <= MAXC chunks per msg tile
                chunk_map = {}      # global col -> (msg tile, local col)
                off16 = 0
                for k, c0, nbk in bm["banks"]:
                    if nbk == 0:
                        continue
                    tab = tables[0] if k < 3 else tables[1]
                    r0 = k * BANK if k < 3 else 0
                    rows = BANK
                    for g0 in range(0, nbk, MAXC):
                        gl = min(MAXC, nbk - g0)
                        mt = msgp.tile([128, gl, DW], FP8, tag="msg")
                        nc.gpsimd.dma_gather(
                            out_ap=mt[:],
                            in_ap=tab[r0 : r0 + rows, :],
                            idxs_ap=it[:, off16 : off16 + gl * 8],
                            num_idxs=gl * CHUNK,
                            num_idxs_reg=nidx_reg(gl * CHUNK),
                            elem_size=DW,
                            single_packet=False,
                            queue_num=k % 4,
                        )
                        for j in range(gl):
                            chunk_map[c0 + g0 + j] = (mt, j)
                        off16 += gl * 8
                return chunk_map, mts, glt

            def agg_layer(idx_dram, tables, bounces, spans, brow_idx,
                          flush, post=None, lookahead=3):
                batches = meta["batches"]
                pending = []
                for bi, tiles in enumerate(batches):
                    pending.append(
                        (bi, tiles,
                         *issue_batch(idx_dram, tables, bounces,
                                      spans, bi, tiles))
                    )
                    if len(pending) > lookahead or bi == len(batches) - 1:
                        todo = pending if bi == len(batches) - 1 else [pending.pop(0)]
                        for job in todo:
                            process_batch(job, brow_idx, flush)
                            if post and job[0] in post:
                                post[job[0]]()
                        if bi == len(batches) - 1:
                            pending = []

            def process_batch(job, brow_idx, flush):
                bi, tiles, chunk_map, mts, glt = job
                bm = meta["batch_meta"][bi]
                for tl, t in enumerate(tiles):
                    acc = accp.tile([128, D], FP32, tag="acc")
                    # self-loop contribution: acc += I @ g_local[tile]
                    chunk_list = bm["tiles"][t]
                    nch_t = len(chunk_list)
                    nc.tensor.matmul(
                        out=acc[:, :],
                        lhsT=ident8_t[:TILE_N, :],
                        rhs=glt[:TILE_N, tl, :D],
                        start=True, stop=(nch_t == 0),
                    )
                    m_t = mts[tl]
                    for ci, (k, col) in enumerate(chunk_list):
                        mt, j = chunk_map[col]
                        nc.tensor.matmul(
                            out=acc[:, :],
                            lhsT=m_t[:, ci, :],
                            rhs=mt[:, j, :D],
                            start=False,
                            stop=(ci + 1 == nch_t),
                        )
                    flush(t, acc)

            def wmm(t, acc, w_t, bias_row):
                """SBUF copy + transpose acc, then (acc @ W) [+ sqdeg x b]"""
                ab = flp.tile([128, D], BF16_T, tag="ab")
                nc.vector.tensor_copy(out=ab[:TILE_N, :], in_=acc[:TILE_N, :])
                abp = tpsp.tile([128, TILE_N], BF16_T, tag="stp")
                nc.tensor.transpose(out=abp[:], in_=ab[:TILE_N, :],
                                    identity=ident_t[:TILE_N, :TILE_N])
                abt = flp.tile([128, 128], BF16_T, tag="stb")
                nc.vector.tensor_copy(out=abt[:, :TILE_N], in_=abp[:])
                hp = gpsp.tile([128, D], FP32, tag="hp")
                nc.tensor.matmul(out=hp[:, :], lhsT=abt[:],
                                 rhs=w_t[:], start=True, stop=not has_bias)
                if has_bias:
                    nc.tensor.matmul(
                        out=hp[:TILE_N, :],
                        lhsT=sqdeg_t[:, t * TILE_N : (t + 1) * TILE_N],
                        rhs=bias_row[:, :],
                        start=False, stop=True,
                    )
                return hp

            # Stage B flush: h1 = relu(dinv*(acc@W1)); table row = dinv*h1
            def flush_b(t, acc):
                dv = dinv_t[:TILE_N, t : t + 1]
                hp = wmm(t, acc, w1_t, brow_ts[0])
                h1 = flp.tile([128, D], BF16_T, tag="h1")
                nc.scalar.activation(out=h1[:TILE_N, :], in_=hp[:TILE_N, :],
                                     func=mybir.ActivationFunctionType.Relu,
                                     scale=dv)
                g2sb = flp.tile([128, DW], FP8, tag="g2sb")
                nc.scalar.activation(out=g2sb[:TILE_N, :D], in_=h1[:TILE_N, :],
                                     func=mybir.ActivationFunctionType.Copy,
                                     scale=dv)
                q = 0 if t < 75 else 1
                r0 = t * TILE_N - (0 if t < 75 else PR[0])
                nc.scalar.dma_start(
                    out=g2_bnc[q][r0 : r0 + TILE_N, :],
                    in_=g2sb[:TILE_N, :],
                )

            # Stage C flush: h2 = relu(dinv*(acc@W2)); logits; log_softmax
            def flush_c(t, acc):
                dv = dinv_t[:TILE_N, t : t + 1]
                hp = wmm(t, acc, w2_t, brow_ts[1])
                h2 = flp.tile([128, D], BF16_T, tag="h1")
                nc.scalar.activation(out=h2[:TILE_N, :], in_=hp[:TILE_N, :],
                                     func=mybir.ActivationFunctionType.Relu,
                                     scale=dv)
                htp = tpsp.tile([128, TILE_N], BF16_T, tag="stp")
                nc.tensor.transpose(out=htp[:], in_=h2[:TILE_N, :],
                                    identity=ident_t[:TILE_N, :TILE_N])
                htb = flp.tile([128, 128], BF16_T, tag="stb")
                nc.vector.tensor_copy(out=htb[:, :TILE_N], in_=htp[:])
                lgf = gpsp.tile([128, D], FP32, tag="hp")
                lg = lgf[:, :DOUT]
                nc.tensor.matmul(out=lg, lhsT=htb[:],
                                 rhs=wfc_t[:, :DOUT], start=True, stop=not has_bias)
                if has_bias:
                    nc.tensor.matmul(out=lgf[:TILE_N, :DOUT],
                                     lhsT=brow_ts[3][:, :TILE_N],
                                     rhs=brow_ts[2][:, :DOUT],
                                     start=False, stop=True)
                mx = flp.tile([128, 1], FP32, tag="mx")
                nc.vector.tensor_reduce(out=mx[:TILE_N, :], in_=lgf[:TILE_N, :DOUT],
                                        axis=mybir.AxisListType.X,
                                        op=mybir.AluOpType.max)
                negm = flp.tile([128, 1], FP32, tag="negm")
                nc.vector.tensor_scalar_mul(out=negm[:TILE_N, :],
                                            in0=mx[:TILE_N, :], scalar1=-1.0)
                esc = flp.tile([128, DOUT], BF16_T, tag="esc")
                ssum = flp.tile([128, 1], FP32, tag="ssum")
                nc.scalar.activation(out=esc[:TILE_N, :], in_=lgf[:TILE_N, :DOUT],
                                     func=mybir.ActivationFunctionType.Exp,
                                     bias=negm[:TILE_N, :],
                                     accum_out=ssum[:TILE_N, :])
                lns = flp.tile([128, 1], FP32, tag="lns")
                nc.scalar.activation(out=lns[:TILE_N, :], in_=ssum[:TILE_N, :],
                                     func=mybir.ActivationFunctionType.Ln)
                nmls = flp.tile([128, 1], FP32, tag="nmls")
                nc.vector.tensor_tensor(out=nmls[:TILE_N, :],
                                        in0=negm[:TILE_N, :],
                                        in1=lns[:TILE_N, :],
                                        op=mybir.AluOpType.subtract)
                ot = flp.tile([128, DOUT], FP32, tag="ot")
                nc.vector.tensor_tensor(out=ot[:TILE_N, :], in0=lgf[:TILE_N, :DOUT],
                                        in1=nmls[:TILE_N, :]
                                            .to_broadcast([TILE_N, DOUT]),
                                        op=mybir.AluOpType.add)
                nc.scalar.dma_start(
                    out=out[t * TILE_N : (t + 1) * TILE_N, :],
                    in_=ot[:TILE_N, :],
                )

            agg_layer(idx1, xgt, xgo, meta["spans1"], 0,
                      flush_b,
                      post={15: lambda: ag(g2_bnc[0], g2_tab[0]),
                            n_batches - 1: lambda: ag(g2_bnc[1], g2_tab[1])})
            agg_layer(idx1, g2_tab, g2_bnc, meta["spans2"], 1,
                      flush_c)

    nc.compile()
    return nc


# ---------------------------------------------------------------------------
# Public entry point
# ---------------------------------------------------------------------------

_CACHE = {}


def kernel(x, edge_index, W1, b1, W2, b2, Wfc, bfc):
    x = np.asarray(x, np.float32)
    per_core, meta, dinv, sqdeg = _preprocess(x, edge_index)
    n, slice_n = meta["n"], meta["slice_n"]

    # same static structure for both layers -> same spans
    idx_packs, mhot_packs = [], []
    spans = None
    fidx = np.arange(128, dtype=np.int16)
    for c in range(N_CORES):
        idx_mat, seg_mat = per_core[c]
        packed, spans = _pack_idx(idx_mat, meta)
        idx_packs.append(packed)
        # host-built one-hot matrices, partition-major [128, n_inst*TILE_N]
        S = seg_mat[meta["inst_cols"], :]               # [n_inst, 128]
        targ = meta["inst_off"][:, None, None] + fidx[None, None, :]
        mh = (S[:, :, None] == targ).astype(F8NP)       # [n_inst, 128, 128]
        mh = np.ascontiguousarray(mh.transpose(1, 0, 2)).reshape(128, -1)
        mhot_packs.append(mh)
    meta["spans1"] = spans
    meta["spans2"] = spans
    s_total = idx_packs[0].shape[1]

    has_bias = bool(
        np.any(np.asarray(b1)) or np.any(np.asarray(b2)) or np.any(np.asarray(bfc))
    )
    key = (n, meta["total_chunks"], s_total, has_bias)
    if key not in _CACHE:
        _CACHE[key] = _build(meta, s_total, has_bias)
    nc = _CACHE[key]

    # full dinv*x gather table in fp8, padded 256B rows, permuted row order
    xg8 = np.zeros((n, 256), F8NP)
    xg8[:, :D] = dinv[:, None] * x
    xgp = np.zeros_like(xg8)
    xgp[meta["perm"], :] = xg8
    P0 = (slice_n // 4) * 3
    xgt0 = np.ascontiguousarray(xgp[: N_CORES * P0])
    xgt1 = np.ascontiguousarray(xgp[N_CORES * P0 :])
    brows = np.zeros((4, D), np.float32)
    brows[0, :] = np.asarray(b1, np.float32)
    brows[1, :] = np.asarray(b2, np.float32)
    brows[2, : DOUT] = np.asarray(bfc, np.float32)
    brows[3, :] = 1.0
    brows = brows.astype(BF16)

    in_maps = []
    for c in range(N_CORES):
        s0 = c * slice_n
        im = {
            "xgt0": xgt0,
            "xgt1": xgt1,
            "xgo0": np.ascontiguousarray(xg8[s0 : s0 + P0]),
            "xgo1": np.ascontiguousarray(xg8[s0 + P0 : s0 + slice_n]),
            "w1": np.asarray(W1, np.float32).astype(BF16),
            "w2": np.asarray(W2, np.float32).astype(BF16),
            "wfc": np.asarray(Wfc, np.float32).astype(BF16),
            "brows": brows,
            "dinvp": _pack_dinv(dinv, s0, meta, np.float32),
            "idx1": idx_packs[c],
            "mhot": mhot_packs[c],
        }
        if has_bias:
            im["sqdegp"] = sqdeg[s0 : s0 + slice_n][None, :].astype(BF16)
        in_maps.append(im)

    global _last_in_maps
    _last_in_maps = in_maps
    last_exc = None
    for _attempt in range(3):
        try:
            res = bass_utils.run_bass_kernel_spmd(
                nc, in_maps, core_ids=list(range(N_CORES))
            )
            return np.concatenate(
                [res.results[c]["out"] for c in range(N_CORES)], axis=0
            )
        except Exception as e:  # transient device/tunnel errors: retry
            last_exc = e
    raise last_exc


_last_in_maps = None


# revision 60
# speedup vs baseline: 1.0452x; 1.0452x over previous
"""Self-contained Trainium2 Bass kernel for a 2-layer GCN + FC + log_softmax.

Distribution: nodes sharded across 8 NeuronCores (12500 rows each); edges
partitioned by destination node so each core's scatter-add is local; gather
tables replicated/exchanged; small weights replicated.

Key structure (per core):
  - Layer 1 aggregates in x-space (A·(dinv*x) then @W1 at flush time, since
    aggregation and the dense matmul commute) so its gather table is a host
    input — no stage-A compute or startup AllGather at all.
  - Aggregation: per 125-dst-node tile, PSUM = ident @ x_own[tile] (self
    loops) + sum over 128-edge chunks onehot.T @ table[src]. Source rows come
    from four 25000-row banks via dma_gather (fp8 rows padded to 256 B — the
    SWDGE descriptor minimum), one bank per SWDGE queue so all four GpSimd
    Q7 pairs generate descriptors concurrently (descriptor generation is the
    critical resource, ~9 ns/row/queue). One-hot matrices are precomputed on
    the host in fp8 (128-wide for fast weight load) and streamed in via HWDGE
    so no compute engine touches them — 2-port DVE work would lock GpSimd
    out of its SBUF descriptor rings.
  - Flush: h = relu(dinv·(acc@W)) (+ rank-1 sqdeg×b if biases are nonzero);
    layer-1 flush emits the fp8 layer-2 table rows (dinv·h1), AllGathered in
    a 3+1 bank split so the big collective hides inside layer 1 and only a
    small one remains at the layer boundary; layer-2 flush applies W2, Wfc
    and a fused log_softmax.
"""
import math

import numpy as np
import ml_dtypes

import concourse.bass as bass
import concourse.mybir as mybir
import concourse.tile as tile
from concourse import bacc, bass_utils

BF16 = ml_dtypes.bfloat16

# Problem contract (hardcoded; must match setup_inputs()).
N_NODES = 100000
N_EDGES = 1600000
D = 128
DOUT = 40

N_CORES = 8
TILE_N = 125            # dst nodes per PSUM tile
BANK = 25000            # gather table bank rows (4 equal banks -> 4 queues)
TB = 5                  # dst tiles per gather batch
CHUNK = 128             # edges per matmul chunk
MAXC = 24               # max chunks per dma_gather call
KM = 24                 # max chunks per one-hot build / matmul run
FP32 = mybir.dt.float32
BF16_T = mybir.dt.bfloat16
I16 = mybir.dt.int16
FP16 = mybir.dt.float16
FP8 = mybir.dt.float8e4
F8NP = mybir.dt.np(mybir.dt.float8e4)
DW = 256                # fp8 table row width (128 data + 128 pad = 256B)


# ---------------------------------------------------------------------------
# Host preprocessing
# ---------------------------------------------------------------------------

def _preprocess(x, edge_index):
    n = x.shape[0]
    slice_n = n // N_CORES
    n_tiles = slice_n // TILE_N
    n_banks = (n + BANK - 1) // BANK

    ei = np.asarray(edge_index, np.int64)
    src = ei[0]
    dst = ei[1]
    # degree includes the self-loops the reference adds
    deg = (np.bincount(dst, minlength=n) + 1).astype(np.float32)
    dinv = 1.0 / np.sqrt(deg)
    sqdeg = np.sqrt(deg)

    # Table rows live in the "split AllGather" layout: part 0 concatenates the
    # cores' first 9375 rows (banks 0-2), part 1 the last 3125 (bank 3) — so
    # banks 0-2 are gatherable after the big mid-layer AllGather and only the
    # small part-1 AllGather remains at the layer boundary.
    P0 = (slice_n // 4) * 3
    P1 = slice_n // 4
    u_ = np.arange(n, dtype=np.int64)
    r_, i_ = u_ // slice_n, u_ % slice_n
    perm_node = np.where(i_ < P0, r_ * P0 + i_,
                         N_CORES * P0 + r_ * P1 + (i_ - P0))
    perm = perm_node[src]

    tile_id = dst // TILE_N                       # global tile (core*n_tiles+t)
    batch_g = (tile_id % n_tiles) // TB           # batch within core
    core_id = tile_id // n_tiles
    bank_id = perm // BANK
    order = np.lexsort((tile_id, bank_id, batch_g, core_id))
    src_s = perm[order]

    batches = [list(range(b, min(b + TB, n_tiles))) for b in range(0, n_tiles, TB)]
    n_batches = len(batches)
    SENT = TB * TILE_N                            # seg sentinel (no tile matches)

    # counts per (core, tile, bank) and per (core, batch, bank)
    key_ctb = (tile_id * n_banks + bank_id)
    cnt_tb = np.bincount(key_ctb, minlength=N_CORES * n_tiles * n_banks).reshape(
        N_CORES, n_tiles, n_banks
    )
    cnt_bb = np.zeros((N_CORES, n_batches, n_banks), np.int64)
    for bi, tiles in enumerate(batches):
        cnt_bb[:, bi, :] = cnt_tb[:, tiles, :].sum(axis=1)
    nbk_bb = ((cnt_bb + CHUNK - 1) // CHUNK).max(axis=0)      # [n_batches, n_banks]

    # group start offsets in the sorted edge array, keyed by (core,batch,bank,tile)
    # sorted order is (core, batch, bank, tile) by the lexsort above
    starts = {}
    pos = 0
    for c in range(N_CORES):
        for bi, tiles in enumerate(batches):
            for k in range(n_banks):
                for t in tiles:
                    m = int(cnt_tb[c, t, k])
                    starts[(c, bi, k, t)] = (pos, m)
                    pos += m
    assert pos == len(src_s)

    # static chunk structure: per batch, per bank, chunk cols; per-tile chunk
    # membership = union over cores
    batch_meta = []
    col = 0
    for bi, tiles in enumerate(batches):
        bm = {"banks": [], "tiles": {t: [] for t in tiles}}
        for k in range(n_banks):
            nbk = int(nbk_bb[bi, k])
            c0 = col
            # per-core tile spans within this (batch, bank) stream
            present = {t: set() for t in tiles}
            for c in range(N_CORES):
                off = 0
                for t in tiles:
                    m = starts[(c, bi, k, t)][1]
                    if m > 0:
                        j0, j1 = off // CHUNK, (off + m - 1) // CHUNK
                        for j in range(j0, j1 + 1):
                            present[t].add(c0 + j)
                    off += m
            for t in tiles:
                for cc in sorted(present[t]):
                    bm["tiles"][t].append((k, cc))
            bm["banks"].append((k, c0, nbk))
            col += nbk
        # order each tile's chunk list by (bank, col) -- already appended that way
        batch_meta.append(bm)
    total_chunks = col

    # Per-core edge arrays in global chunk-column order.
    per_core = []
    for c in range(N_CORES):
        idx_mat = np.zeros((total_chunks, CHUNK), np.int16)
        seg_mat = np.full((total_chunks, CHUNK), SENT, np.int16)
        for bi, tiles in enumerate(batches):
            bm = batch_meta[bi]
            node0 = tiles[0] * TILE_N            # batch-local dst base (per-core)
            for k, c0, nbk in bm["banks"]:
                parts_i, parts_s = [], []
                for t in tiles:
                    s0, m = starts[(c, bi, k, t)]
                    parts_i.append(src_s[s0 : s0 + m] - k * BANK)
                    parts_s.append(dst[order[s0 : s0 + m]] - c * slice_n - node0)
                eidx = np.concatenate(parts_i).astype(np.int16)
                es = np.concatenate(parts_s).astype(np.int16)
                m = len(eidx)
                pad = nbk * CHUNK - m
                eidx = np.concatenate([eidx, np.zeros(pad, np.int16)])
                es = np.concatenate([es, np.full(pad, SENT, np.int16)])
                idx_mat[c0 : c0 + nbk] = eidx.reshape(nbk, CHUNK)
                seg_mat[c0 : c0 + nbk] = es.reshape(nbk, CHUNK)
        per_core.append((idx_mat, seg_mat))

    # one-hot instance table: per (batch, tile), one instance per chunk in the
    # tile's chunk list, contiguous in (batch, tile, chunk) order
    inst_start = {}
    inst_cols = []
    inst_off = []
    for bi, tiles in enumerate(batches):
        bm = batch_meta[bi]
        for tl, t in enumerate(tiles):
            inst_start[(bi, t)] = len(inst_cols)
            for k, cc in bm["tiles"][t]:
                inst_cols.append(cc)
                inst_off.append(tl * TILE_N)
    maxi = max(len(batch_meta[bi]["tiles"][t])
               for bi, tiles in enumerate(batches) for t in tiles)

    meta = {
        "n": n, "slice_n": slice_n, "n_tiles": n_tiles, "n_banks": n_banks,
        "batches": batches, "batch_meta": batch_meta, "total_chunks": total_chunks,
        "perm": perm_node,
        "inst_start": inst_start, "inst_cols": np.array(inst_cols),
        "inst_off": np.array(inst_off), "n_inst": len(inst_cols), "maxi": maxi,
    }
    return per_core, meta, dinv, sqdeg


def _pack_idx(idx_mat, meta):
    """Wrap chunk-major indices into the dma_gather [16, n/16] layout per
    (batch, bank) gather call, concatenated along the free dim, replicated to
    128 partitions. Returns ([128, S_total] int16, per-batch [start, len])."""
    spans = []
    blocks = []
    s = 0
    for bm in meta["batch_meta"]:
        s0 = s
        for k, c0, nbk in bm["banks"]:
            if nbk == 0:
                continue
            for g0 in range(0, nbk, MAXC):
                gl = min(MAXC, nbk - g0)
                flat = idx_mat[c0 + g0 : c0 + g0 + gl].reshape(-1)   # [gl*128]
                wrapped = flat.reshape(-1, 16).T                     # [16, gl*8]
                blocks.append(wrapped)
                s += wrapped.shape[1]
        spans.append((s0, s - s0))
    packed = np.concatenate(blocks, axis=1) if blocks else np.zeros((16, 0), np.int16)
    packed = np.tile(packed, (8, 1)).copy()                    # [128, S]
    return packed, spans


def _pack_dinv(v, slice0, meta, dtype):
    """[128, n_tiles]: partition p, col t = v[slice0 + t*TILE_N + p] (p<125)."""
    n_tiles = meta["n_tiles"]
    out = np.zeros((128, n_tiles), dtype)
    sl = v[slice0 : slice0 + n_tiles * TILE_N].reshape(n_tiles, TILE_N)
    out[:TILE_N, :] = sl.T
    return out


# ---------------------------------------------------------------------------
# Device kernel builder
# ---------------------------------------------------------------------------

def _build(meta, s_total, has_bias):
    n = meta["n"]
    slice_n = meta["slice_n"]
    n_tiles = meta["n_tiles"]
    n_banks = meta["n_banks"]
    total_chunks = meta["total_chunks"]
    n_a_tiles = math.ceil(slice_n / 128)

    nc = bacc.Bacc("TRN2", target_bir_lowering=False, debug=False,
                   num_devices=N_CORES, num_swdge_queues=4,
                   dynamic_dma_scratch_size=32768)

    # inputs
    xgt = [None, None]   # full dinv*x gather table, permuted, split 3+1
    xgo = [None, None]   # this core's own dinv*x rows (for self-loops)
    w1 = nc.dram_tensor("w1", [D, D], BF16_T, kind="ExternalInput")
    w2 = nc.dram_tensor("w2", [D, D], BF16_T, kind="ExternalInput")
    wfc = nc.dram_tensor("wfc", [D, DOUT], BF16_T, kind="ExternalInput")
    brows = nc.dram_tensor("brows", [4, D], BF16_T, kind="ExternalInput")
    # brows rows: 0=b1, 1=b2, 2=bfc (padded), 3=ones
    dinvp = nc.dram_tensor("dinvp", [128, n_tiles], FP32, kind="ExternalInput")
    sqdegp = (nc.dram_tensor("sqdegp", [1, slice_n], BF16_T, kind="ExternalInput")
              if has_bias else None)
    idx1 = nc.dram_tensor("idx1", [128, s_total], I16, kind="ExternalInput")
    mhot = nc.dram_tensor("mhot", [128, meta["n_inst"] * 128], FP8,
                          kind="ExternalInput")
    maxi = meta["maxi"]

    out = nc.dram_tensor("out", [slice_n, DOUT], FP32, kind="ExternalOutput")

    # internal dram — tables and bounces split 3+1: part 0 = banks 0-2
    # (9375 rows/core), part 1 = bank 3 (3125 rows/core)
    PR = [(slice_n // 4) * 3, slice_n // 4]
    NP_ = [N_CORES * PR[0], N_CORES * PR[1]]
    for q in range(2):
        xgt[q] = nc.dram_tensor(f"xgt{q}", [NP_[q], DW], FP8, kind="ExternalInput")
        xgo[q] = nc.dram_tensor(f"xgo{q}", [PR[q], DW], FP8, kind="ExternalInput")
    g2_bnc = [nc.dram_tensor(f"g2_bounce{q}", [PR[q], DW], FP8) for q in range(2)]
    g2_tab = [nc.dram_tensor(f"g2_table{q}", [NP_[q], DW], FP8, addr_space="Shared")
              for q in range(2)]

    with tile.TileContext(nc) as tc:
        with (
            tc.tile_pool(name="const", bufs=1) as constp,
            tc.tile_pool(name="msg", bufs=12) as msgp,
            tc.tile_pool(name="idxp", bufs=6) as idxp,
            tc.tile_pool(name="glp", bufs=4) as glp,
            tc.tile_pool(name="mp", bufs=8) as mp,
            tc.tile_pool(name="fl", bufs=4) as flp,
            tc.tile_pool(name="acc", bufs=3, space="PSUM") as accp,
            tc.tile_pool(name="tps", bufs=2, space="PSUM") as tpsp,
            tc.tile_pool(name="gps", bufs=2, space="PSUM") as gpsp,
        ):
            # constants
            w1_t = constp.tile([D, D], BF16_T, tag="w1")
            nc.sync.dma_start(out=w1_t[:], in_=w1[:, :])
            w2_t = constp.tile([D, D], BF16_T, tag="w2")
            nc.sync.dma_start(out=w2_t[:], in_=w2[:, :])
            wfc_t = constp.tile([D, DOUT], BF16_T, tag="wfc")
            nc.sync.dma_start(out=wfc_t[:], in_=wfc[:, :])
            brow_ts = []
            for r in range(4):
                bt = constp.tile([1, D], BF16_T, tag=f"brow{r}")
                nc.sync.dma_start(out=bt[:], in_=brows[r : r + 1, :])
                brow_ts.append(bt)
            dinv_t = constp.tile([128, n_tiles], FP32, tag="dinvp")
            nc.sync.dma_start(out=dinv_t[:], in_=dinvp[:, :])
            if has_bias:
                sqdeg_t = constp.tile([1, slice_n], BF16_T, tag="sqdegp")
                nc.sync.dma_start(out=sqdeg_t[:], in_=sqdegp[:, :])
            ident_t = constp.tile([128, 128], BF16_T, tag="ident")
            from concourse.masks import make_identity
            make_identity(nc, ident_t[:])
            ident8_t = constp.tile([128, 128], FP8, tag="ident8")
            make_identity(nc, ident8_t[:])

            # ---------------- Stage A ----------------
            # one bulk input load per half; matmuls slice it; grouped stores
            def ag(bounce_h, table_h):
                nc.gpsimd.collective_compute(
                    "AllGather", mybir.AluOpType.bypass,
                    ins=[bounce_h[:, :]], outs=[table_h[:, :]],
                    replica_groups=[list(range(N_CORES))],
                )

            # ---------------- aggregation layers ----------------
            idx_all = constp.tile([128, s_total], I16, tag="idxall")
            nc.sync.dma_start(out=idx_all[:], in_=idx1[:, :])
            _regs = {}

            def nidx_reg(v):
                if v not in _regs:
                    _regs[v] = nc.gpsimd.to_reg(v)
                return _regs[v]

            n_batches = len(meta["batches"])
            per_q = n_batches // 4

            def issue_batch(idx_dram, tables, bounces, spans, bi, tiles):
                bm = meta["batch_meta"][bi]
                s0, slen = spans[bi]
                it = idx_all[:, s0 : s0 + slen]

                # host-precomputed one-hot tiles, one per dst tile
                mts = []
                for ti, t in enumerate(tiles):
                    i0 = meta["inst_start"][(bi, t)]
                    ni = len(bm["tiles"][t])
                    m_t = mp.tile([128, maxi, 128], FP8, tag="m")
                    eng = nc.sync if ti % 2 == 0 else nc.scalar
                    eng.dma_start(
                        out=m_t[:, :ni, :],
                        in_=mhot[:, i0 * 128 : (i0 + ni) * 128]
                            .rearrange("p (i f) -> p i f", f=128),
                    )
                    mts.append(m_t)

                # local g rows for the batch's self-loop matmuls:
                # glt[p, j, :] = bounce_half[row0 + j*TILE_N + p, :]
                ntl = len(tiles)
                q = 0 if (tiles[-1] + 1) * TILE_N <= PR[0] else 1
                row0 = tiles[0] * TILE_N - (0 if q == 0 else PR[0])
                glt = glp.tile([128, ntl, DW], FP8, tag="gl")
                nc.scalar.dma_start(
                    out=glt[:TILE_N, :, :],
                    in_=bounces[q][row0 : row0 + ntl * TILE_N, :]
                        .rearrange("(t p) d -> p t d", p=TILE_N),
                )

                # bank gathers, split to <= MAXC chunks per msg tile
                chunk_map = {}      # global col -> (msg tile, local col)
                off16 = 0
                for k, c0, nbk in bm["banks"]:
                    if nbk == 0:
                        continue
                    tab = tables[0] if k < 3 else tables[1]
                    r0 = k * BANK if k < 3 else 0
                    rows = BANK
                    for g0 in range(0, nbk, MAXC):
                        gl = min(MAXC, nbk - g0)
                        mt = msgp.tile([128, gl, DW], FP8, tag="msg")
                        nc.gpsimd.dma_gather(
                            out_ap=mt[:],
                            in_ap=tab[r0 : r0 + rows, :],
                            idxs_ap=it[:, off16 : off16 + gl * 8],
                            num_idxs=gl * CHUNK,
                            num_idxs_reg=nidx_reg(gl * CHUNK),
                            elem_size=DW,
                            single_packet=False,
                            queue_num=k % 4,
                        )
                        for j in range(gl):
                            chunk_map[c0 + g0 + j] = (mt, j)
                        off16 += gl * 8
                return chunk_map, mts, glt

            def agg_layer(idx_dram, tables, bounces, spans, brow_idx,
                          flush, post=None, lookahead=3):
                batches = meta["batches"]
                pending = []
                for bi, tiles in enumerate(batches):
                    pending.append(
                        (bi, tiles,
                         *issue_batch(idx_dram, tables, bounces,
                                      spans, bi, tiles))
                    )
                    if len(pending) > lookahead or bi == len(batches) - 1:
                        todo = pending if bi == len(batches) - 1 else [pending.pop(0)]
                        for job in todo:
                            process_batch(job, brow_idx, flush)
                            if post and job[0] in post:
                                post[job[0]]()
                        if bi == len(batches) - 1:
                            pending = []

            def process_batch(job, brow_idx, flush):
                bi, tiles, chunk_map, mts, glt = job
                bm = meta["batch_meta"][bi]
                for tl, t in enumerate(tiles):
                    acc = accp.tile([128, D], FP32, tag="acc")
                    # self-loop contribution: acc += I @ g_local[tile]
                    chunk_list = bm["tiles"][t]
                    nch_t = len(chunk_list)
                    nc.tensor.matmul(
                        out=acc[:, :],
                        lhsT=ident8_t[:TILE_N, :],
                        rhs=glt[:TILE_N, tl, :D],
                        start=True, stop=(nch_t == 0),
                    )
                    m_t = mts[tl]
                    for ci, (k, col) in enumerate(chunk_list):
                        mt, j = chunk_map[col]
                        nc.tensor.matmul(
                            out=acc[:, :],
                            lhsT=m_t[:, ci, :],
                            rhs=mt[:, j, :D],
                            start=False,
                            stop=(ci + 1 == nch_t),
                        )
                    flush(t, acc)

            def wmm(t, acc, w_t, bias_row):
                """SBUF copy + transpose acc, then (acc @ W) [+ sqdeg x b]"""
                ab = flp.tile([128, D], BF16_T, tag="ab")
                nc.vector.tensor_copy(out=ab[:TILE_N, :], in_=acc[:TILE_N, :])
                abp = tpsp.tile([128, TILE_N], BF16_T, tag="stp")
                nc.tensor.transpose(out=abp[:], in_=ab[:TILE_N, :],
                                    identity=ident_t[:TILE_N, :TILE_N])
                abt = flp.tile([128, 128], BF16_T, tag="stb")
                nc.vector.tensor_copy(out=abt[:, :TILE_N], in_=abp[:])
                hp = gpsp.tile([128, D], FP32, tag="hp")
                nc.tensor.matmul(out=hp[:, :], lhsT=abt[:],
                                 rhs=w_t[:], start=True, stop=not has_bias)
                if has_bias:
                    nc.tensor.matmul(
                        out=hp[:TILE_N, :],
                        lhsT=sqdeg_t[:, t * TILE_N : (t + 1) * TILE_N],
                        rhs=bias_row[:, :],
                        start=False, stop=True,
                    )
                return hp

            # Stage B flush: h1 = relu(dinv*(acc@W1)); table row = dinv*h1
            def flush_b(t, acc):
                dv = dinv_t[:TILE_N, t : t + 1]
                hp = wmm(t, acc, w1_t, brow_ts[0])
                h1 = flp.tile([128, D], BF16_T, tag="h1")
                nc.scalar.activation(out=h1[:TILE_N, :], in_=hp[:TILE_N, :],
                                     func=mybir.ActivationFunctionType.Relu,
                                     scale=dv)
                g2sb = flp.tile([128, DW], FP8, tag="g2sb")
                nc.scalar.activation(out=g2sb[:TILE_N, :D], in_=h1[:TILE_N, :],
                                     func=mybir.ActivationFunctionType.Copy,
                                     scale=dv)
                q = 0 if t < 75 else 1
                r0 = t * TILE_N - (0 if t < 75 else PR[0])
                nc.scalar.dma_start(
                    out=g2_bnc[q][r0 : r0 + TILE_N, :],
                    in_=g2sb[:TILE_N, :],
                )

            # Stage C flush: h2 = relu(dinv*(acc@W2)); logits; log_softmax
            def flush_c(t, acc):
                dv = dinv_t[:TILE_N, t : t + 1]
                hp = wmm(t, acc, w2_t, brow_ts[1])
                h2 = flp.tile([128, D], BF16_T, tag="h1")
                nc.scalar.activation(out=h2[:TILE_N, :], in_=hp[:TILE_N, :],
                                     func=mybir.ActivationFunctionType.Relu,
                                     scale=dv)
                htp = tpsp.tile([128, TILE_N], BF16_T, tag="stp")
                nc.tensor.transpose(out=htp[:], in_=h2[:TILE_N, :],
                                    identity=ident_t[:TILE_N, :TILE_N])
                htb = flp.tile([128, 128], BF16_T, tag="stb")
                nc.vector.tensor_copy(out=htb[:, :TILE_N], in_=htp[:])
                lgf = gpsp.tile([128, D], FP32, tag="hp")
                lg = lgf[:, :DOUT]
                nc.tensor.matmul(out=lg, lhsT=htb[:],
                                 rhs=wfc_t[:, :DOUT], start=True, stop=not has_bias)
                if has_bias:
                    nc.tensor.matmul(out=lgf[:TILE_N, :DOUT],
                                     lhsT=brow_ts[3][:, :TILE_N],
                                     rhs=brow_ts[2][:, :DOUT],
                                     start=False, stop=True)
                mx = flp.tile([128, 1], FP32, tag="mx")
                nc.vector.tensor_reduce(out=mx[:TILE_N, :], in_=lgf[:TILE_N, :DOUT],
                                        axis=mybir.AxisListType.X,
                                        op=mybir.AluOpType.max)
                negm = flp.tile([128, 1], FP32, tag="negm")
                nc.vector.tensor_scalar_mul(out=negm[:TILE_N, :],
                                            in0=mx[:TILE_N, :], scalar1=-1.0)
                esc = flp.tile([128, DOUT], BF16_T, tag="esc")
                ssum = flp.tile([128, 1], FP32, tag="ssum")
                nc.scalar.activation(out=esc[:TILE_N, :], in_=lgf[:TILE_N, :DOUT],
                                     func=mybir.ActivationFunctionType.Exp,
                                     bias=negm[:TILE_N, :],
                                     accum_out=ssum[:TILE_N, :])
                lns = flp.tile([128, 1], FP32, tag="lns")
                nc.scalar.activation(out=lns[:TILE_N, :], in_=ssum[:TILE_N, :],
                                     func=mybir.ActivationFunctionType.Ln)
                nmls = flp.tile([128, 1], FP32, tag="nmls")
                nc.vector.tensor_tensor(out=nmls[:TILE_N, :],
                                        in0=negm[:TILE_N, :],
                                        in1=lns[:TILE_N, :],
                                        op=mybir.AluOpType.subtract)
                ot = flp.tile([128, DOUT], FP32, tag="ot")
                nc.vector.tensor_tensor(out=ot[:TILE_N, :], in0=lgf[:TILE_N, :DOUT],
                                        in1=nmls[:TILE_N, :]
                                            .to_broadcast([TILE_N, DOUT]),
                                        op=mybir.AluOpType.add)
                nc.scalar.dma_start(
                    out=out[t * TILE_N : (t + 1) * TILE_N, :],
                    in_=ot[:TILE_N, :],
                )

            agg_layer(idx1, xgt, xgo, meta["spans1"], 0,
                      flush_b,
                      post={15: lambda: ag(g2_bnc[0], g2_tab[0]),
                            n_batches - 1: lambda: ag(g2_bnc[1], g2_tab[1])})
            agg_layer(idx1, g2_tab, g2_bnc, meta["spans2"], 1,
                      flush_c)

    nc.compile()
    return nc


# ---------------------------------------------------------------------------
# Public entry point
# ---------------------------------------------------------------------------

_CACHE = {}


def kernel(x, edge_index, W1, b1, W2, b2, Wfc, bfc):
    x = np.asarray(x, np.float32)
    per_core, meta, dinv, sqdeg = _preprocess(x, edge_index)
    n, slice_n = meta["n"], meta["slice_n"]

    # same static structure for both layers -> same spans
    idx_packs, mhot_packs = [], []
    spans = None
    fidx = np.arange(128, dtype=np.int16)
    for c in range(N_CORES):
        idx_mat, seg_mat = per_core[c]
        packed, spans = _pack_idx(idx_mat, meta)
        idx_packs.append(packed)
        # host-built one-hot matrices, partition-major [128, n_inst*TILE_N]
        S = seg_mat[meta["inst_cols"], :]               # [n_inst, 128]
        targ = meta["inst_off"][:, None, None] + fidx[None, None, :]
        mh = (S[:, :, None] == targ).astype(F8NP)       # [n_inst, 128, 128]
        mh = np.ascontiguousarray(mh.transpose(1, 0, 2)).reshape(128, -1)
        mhot_packs.append(mh)
    meta["spans1"] = spans
    meta["spans2"] = spans
    s_total = idx_packs[0].shape[1]

    has_bias = bool(
        np.any(np.asarray(b1)) or np.any(np.asarray(b2)) or np.any(np.asarray(bfc))
    )
    key = (n, meta["total_chunks"], s_total, has_bias)
    if key not in _CACHE:
        _CACHE[key] = _build(meta, s_total, has_bias)
    nc = _CACHE[key]

    # full dinv*x gather table in fp8, padded 256B rows, permuted row order
    xg8 = np.zeros((n, 256), F8NP)
    xg8[:, :D] = dinv[:, None] * x
    xgp = np.zeros_like(xg8)
    xgp[meta["perm"], :] = xg8
    P0 = (slice_n // 4) * 3
    xgt0 = np.ascontiguousarray(xgp[: N_CORES * P0])
    xgt1 = np.ascontiguousarray(xgp[N_CORES * P0 :])
    brows = np.zeros((4, D), np.float32)
    brows[0, :] = np.asarray(b1, np.float32)
    brows[1, :] = np.asarray(b2, np.float32)
    brows[2, : DOUT] = np.asarray(bfc, np.float32)
    brows[3, :] = 1.0
    brows = brows.astype(BF16)

    in_maps = []
    for c in range(N_CORES):
        s0 = c * slice_n
        im = {
            "xgt0": xgt0,
            "xgt1": xgt1,
            "xgo0": np.ascontiguousarray(xg8[s0 : s0 + P0]),
            "xgo1": np.ascontiguousarray(xg8[s0 + P0 : s0 + slice_n]),
            "w1": np.asarray(W1, np.float32).astype(BF16),
            "w2": np.asarray(W2, np.float32).astype(BF16),
            "wfc": np.asarray(Wfc, np.float32).astype(BF16),
            "brows": brows,
            "dinvp": _pack_dinv(dinv, s0, meta, np.float32),
            "idx1": idx_packs[c],
            "mhot": mhot_packs[c],
        }
        if has_bias:
            im["sqdegp"] = sqdeg[s0 : s0 + slice_n][None, :].astype(BF16)
        in_maps.append(im)

    global _last_in_maps
    _last_in_maps = in_maps
    last_exc = None
    for _attempt in range(3):
        try:
            res = bass_utils.run_bass_kernel_spmd(
                nc, in_maps, core_ids=list(range(N_CORES))
            )
            return np.concatenate(
                [res.results[c]["out"] for c in range(N_CORES)], axis=0
            )
        except Exception as e:  # transient device/tunnel errors: retry
            last_exc = e
    raise last_exc


_last_in_maps = None


# revision 61
# speedup vs baseline: 1.0569x; 1.0113x over previous
"""Self-contained Trainium2 Bass kernel for a 2-layer GCN + FC + log_softmax.

Distribution: nodes sharded across 8 NeuronCores (12500 rows each); edges
partitioned by destination node so each core's scatter-add is local; gather
tables replicated/exchanged; small weights replicated.

Key structure (per core):
  - Layer 1 aggregates in x-space (A·(dinv*x) then @W1 at flush time, since
    aggregation and the dense matmul commute) so its gather table is a host
    input — no stage-A compute or startup AllGather at all.
  - Aggregation: per 125-dst-node tile, PSUM = ident @ x_own[tile] (self
    loops) + sum over 128-edge chunks onehot.T @ table[src]. Source rows come
    from four 25000-row banks via dma_gather (fp8 rows padded to 256 B — the
    SWDGE descriptor minimum), one bank per SWDGE queue so all four GpSimd
    Q7 pairs generate descriptors concurrently (descriptor generation is the
    critical resource, ~9 ns/row/queue). One-hot matrices are precomputed on
    the host in fp8 (128-wide for fast weight load) and streamed in via HWDGE
    so no compute engine touches them — 2-port DVE work would lock GpSimd
    out of its SBUF descriptor rings.
  - Flush: h = relu(dinv·(acc@W)) (+ rank-1 sqdeg×b if biases are nonzero);
    layer-1 flush emits the fp8 layer-2 table rows (dinv·h1), AllGathered in
    a 3+1 bank split so the big collective hides inside layer 1 and only a
    small one remains at the layer boundary; layer-2 flush applies W2, Wfc
    and a fused log_softmax.
"""
import math

import numpy as np
import ml_dtypes

import concourse.bass as bass
import concourse.mybir as mybir
import concourse.tile as tile
from concourse import bacc, bass_utils

BF16 = ml_dtypes.bfloat16

# Problem contract (hardcoded; must match setup_inputs()).
N_NODES = 100000
N_EDGES = 1600000
D = 128
DOUT = 40

N_CORES = 8
TILE_N = 125            # dst nodes per PSUM tile
BANK = 25000            # gather table bank rows (4 equal banks -> 4 queues)
TB = 5                  # dst tiles per gather batch
CHUNK = 128             # edges per matmul chunk
MAXC = 24               # max chunks per dma_gather call
KM = 24                 # max chunks per one-hot build / matmul run
FP32 = mybir.dt.float32
BF16_T = mybir.dt.bfloat16
I16 = mybir.dt.int16
FP16 = mybir.dt.float16
FP8 = mybir.dt.float8e4
F8NP = mybir.dt.np(mybir.dt.float8e4)
DW = 256                # fp8 table row width (128 data + 128 pad = 256B)


# ---------------------------------------------------------------------------
# Host preprocessing
# ---------------------------------------------------------------------------

def _preprocess(x, edge_index):
    n = x.shape[0]
    slice_n = n // N_CORES
    n_tiles = slice_n // TILE_N
    n_banks = (n + BANK - 1) // BANK

    ei = np.asarray(edge_index, np.int64)
    src = ei[0]
    dst = ei[1]
    # degree includes the self-loops the reference adds
    deg = (np.bincount(dst, minlength=n) + 1).astype(np.float32)
    dinv = 1.0 / np.sqrt(deg)
    sqdeg = np.sqrt(deg)

    # Table rows live in the "split AllGather" layout: part 0 concatenates the
    # cores' first 9375 rows (banks 0-2), part 1 the last 3125 (bank 3) — so
    # banks 0-2 are gatherable after the big mid-layer AllGather and only the
    # small part-1 AllGather remains at the layer boundary.
    P0 = (slice_n // 4) * 3
    P1 = slice_n // 4
    u_ = np.arange(n, dtype=np.int64)
    r_, i_ = u_ // slice_n, u_ % slice_n
    perm_node = np.where(i_ < P0, r_ * P0 + i_,
                         N_CORES * P0 + r_ * P1 + (i_ - P0))
    perm = perm_node[src]

    tile_id = dst // TILE_N                       # global tile (core*n_tiles+t)
    batch_g = (tile_id % n_tiles) // TB           # batch within core
    core_id = tile_id // n_tiles
    bank_id = perm // BANK
    order = np.lexsort((tile_id, bank_id, batch_g, core_id))
    src_s = perm[order]

    batches = [list(range(b, min(b + TB, n_tiles))) for b in range(0, n_tiles, TB)]
    n_batches = len(batches)
    SENT = TB * TILE_N                            # seg sentinel (no tile matches)

    # counts per (core, tile, bank) and per (core, batch, bank)
    key_ctb = (tile_id * n_banks + bank_id)
    cnt_tb = np.bincount(key_ctb, minlength=N_CORES * n_tiles * n_banks).reshape(
        N_CORES, n_tiles, n_banks
    )
    cnt_bb = np.zeros((N_CORES, n_batches, n_banks), np.int64)
    for bi, tiles in enumerate(batches):
        cnt_bb[:, bi, :] = cnt_tb[:, tiles, :].sum(axis=1)
    nbk_bb = ((cnt_bb + CHUNK - 1) // CHUNK).max(axis=0)      # [n_batches, n_banks]

    # group start offsets in the sorted edge array, keyed by (core,batch,bank,tile)
    # sorted order is (core, batch, bank, tile) by the lexsort above
    starts = {}
    pos = 0
    for c in range(N_CORES):
        for bi, tiles in enumerate(batches):
            for k in range(n_banks):
                for t in tiles:
                    m = int(cnt_tb[c, t, k])
                    starts[(c, bi, k, t)] = (pos, m)
                    pos += m
    assert pos == len(src_s)

    # static chunk structure: per batch, per bank, chunk cols; per-tile chunk
    # membership = union over cores
    batch_meta = []
    col = 0
    for bi, tiles in enumerate(batches):
        bm = {"banks": [], "tiles": {t: [] for t in tiles}}
        for k in range(n_banks):
            nbk = int(nbk_bb[bi, k])
            c0 = col
            # per-core tile spans within this (batch, bank) stream
            present = {t: set() for t in tiles}
            for c in range(N_CORES):
                off = 0
                for t in tiles:
                    m = starts[(c, bi, k, t)][1]
                    if m > 0:
                        j0, j1 = off // CHUNK, (off + m - 1) // CHUNK
                        for j in range(j0, j1 + 1):
                            present[t].add(c0 + j)
                    off += m
            for t in tiles:
                for cc in sorted(present[t]):
                    bm["tiles"][t].append((k, cc))
            bm["banks"].append((k, c0, nbk))
            col += nbk
        # order each tile's chunk list by (bank, col) -- already appended that way
        batch_meta.append(bm)
    total_chunks = col

    # Per-core edge arrays in global chunk-column order.
    per_core = []
    for c in range(N_CORES):
        idx_mat = np.zeros((total_chunks, CHUNK), np.int16)
        seg_mat = np.full((total_chunks, CHUNK), SENT, np.int16)
        for bi, tiles in enumerate(batches):
            bm = batch_meta[bi]
            node0 = tiles[0] * TILE_N            # batch-local dst base (per-core)
            for k, c0, nbk in bm["banks"]:
                parts_i, parts_s = [], []
                for t in tiles:
                    s0, m = starts[(c, bi, k, t)]
                    parts_i.append(src_s[s0 : s0 + m] - k * BANK)
                    parts_s.append(dst[order[s0 : s0 + m]] - c * slice_n - node0)
                eidx = np.concatenate(parts_i).astype(np.int16)
                es = np.concatenate(parts_s).astype(np.int16)
                m = len(eidx)
                pad = nbk * CHUNK - m
                eidx = np.concatenate([eidx, np.zeros(pad, np.int16)])
                es = np.concatenate([es, np.full(pad, SENT, np.int16)])
                idx_mat[c0 : c0 + nbk] = eidx.reshape(nbk, CHUNK)
                seg_mat[c0 : c0 + nbk] = es.reshape(nbk, CHUNK)
        per_core.append((idx_mat, seg_mat))

    # one-hot instance table: per (batch, tile), one instance per chunk in the
    # tile's chunk list, contiguous in (batch, tile, chunk) order
    inst_start = {}
    inst_cols = []
    inst_off = []
    for bi, tiles in enumerate(batches):
        bm = batch_meta[bi]
        for tl, t in enumerate(tiles):
            inst_start[(bi, t)] = len(inst_cols)
            for k, cc in bm["tiles"][t]:
                inst_cols.append(cc)
                inst_off.append(tl * TILE_N)
    maxi = max(len(batch_meta[bi]["tiles"][t])
               for bi, tiles in enumerate(batches) for t in tiles)

    meta = {
        "n": n, "slice_n": slice_n, "n_tiles": n_tiles, "n_banks": n_banks,
        "batches": batches, "batch_meta": batch_meta, "total_chunks": total_chunks,
        "perm": perm_node,
        "inst_start": inst_start, "inst_cols": np.array(inst_cols),
        "inst_off": np.array(inst_off), "n_inst": len(inst_cols), "maxi": maxi,
    }
    return per_core, meta, dinv, sqdeg


def _pack_idx(idx_mat, meta):
    """Wrap chunk-major indices into the dma_gather [16, n/16] layout per
    (batch, bank) gather call, concatenated along the free dim, replicated to
    128 partitions. Returns ([128, S_total] int16, per-batch [start, len])."""
    spans = []
    blocks = []
    s = 0
    for bm in meta["batch_meta"]:
        s0 = s
        for k, c0, nbk in bm["banks"]:
            if nbk == 0:
                continue
            for g0 in range(0, nbk, MAXC):
                gl = min(MAXC, nbk - g0)
                flat = idx_mat[c0 + g0 : c0 + g0 + gl].reshape(-1)   # [gl*128]
                wrapped = flat.reshape(-1, 16).T                     # [16, gl*8]
                blocks.append(wrapped)
                s += wrapped.shape[1]
        spans.append((s0, s - s0))
    packed = np.concatenate(blocks, axis=1) if blocks else np.zeros((16, 0), np.int16)
    packed = np.tile(packed, (8, 1)).copy()                    # [128, S]
    return packed, spans


def _pack_dinv(v, slice0, meta, dtype):
    """[128, n_tiles]: partition p, col t = v[slice0 + t*TILE_N + p] (p<125)."""
    n_tiles = meta["n_tiles"]
    out = np.zeros((128, n_tiles), dtype)
    sl = v[slice0 : slice0 + n_tiles * TILE_N].reshape(n_tiles, TILE_N)
    out[:TILE_N, :] = sl.T
    return out


# ---------------------------------------------------------------------------
# Device kernel builder
# ---------------------------------------------------------------------------

def _build(meta, s_total, has_bias):
    n = meta["n"]
    slice_n = meta["slice_n"]
    n_tiles = meta["n_tiles"]
    n_banks = meta["n_banks"]
    total_chunks = meta["total_chunks"]
    n_a_tiles = math.ceil(slice_n / 128)

    nc = bacc.Bacc("TRN2", target_bir_lowering=False, debug=False,
                   num_devices=N_CORES, num_swdge_queues=4,
                   dynamic_dma_scratch_size=32768)

    # inputs
    xgt = [None, None]   # full dinv*x gather table, permuted, split 3+1
    xgo = [None, None]   # this core's own dinv*x rows (for self-loops)
    w1 = nc.dram_tensor("w1", [D, D], BF16_T, kind="ExternalInput")
    w2 = nc.dram_tensor("w2", [D, D], BF16_T, kind="ExternalInput")
    wfc = nc.dram_tensor("wfc", [D, DOUT], BF16_T, kind="ExternalInput")
    brows = nc.dram_tensor("brows", [4, D], BF16_T, kind="ExternalInput")
    # brows rows: 0=b1, 1=b2, 2=bfc (padded), 3=ones
    dinvp = nc.dram_tensor("dinvp", [128, n_tiles], FP32, kind="ExternalInput")
    sqdegp = (nc.dram_tensor("sqdegp", [1, slice_n], BF16_T, kind="ExternalInput")
              if has_bias else None)
    idx1 = nc.dram_tensor("idx1", [128, s_total], I16, kind="ExternalInput")
    mhot = nc.dram_tensor("mhot", [128, meta["n_inst"] * 128], FP8,
                          kind="ExternalInput")
    maxi = meta["maxi"]

    out = nc.dram_tensor("out", [slice_n, DOUT], FP32, kind="ExternalOutput")

    # internal dram — tables and bounces split 3+1: part 0 = banks 0-2
    # (9375 rows/core), part 1 = bank 3 (3125 rows/core)
    PR = [(slice_n // 4) * 3, slice_n // 4]
    NP_ = [N_CORES * PR[0], N_CORES * PR[1]]
    for q in range(2):
        xgt[q] = nc.dram_tensor(f"xgt{q}", [NP_[q], DW], FP8, kind="ExternalInput")
        xgo[q] = nc.dram_tensor(f"xgo{q}", [PR[q], DW], FP8, kind="ExternalInput")
    g2_bnc = [nc.dram_tensor(f"g2_bounce{q}", [PR[q], DW], FP8) for q in range(2)]
    g2_tab = [nc.dram_tensor(f"g2_table{q}", [NP_[q], DW], FP8, addr_space="Shared")
              for q in range(2)]

    with tile.TileContext(nc) as tc:
        with (
            tc.tile_pool(name="const", bufs=1) as constp,
            tc.tile_pool(name="msg", bufs=12) as msgp,
            tc.tile_pool(name="idxp", bufs=6) as idxp,
            tc.tile_pool(name="glp", bufs=4) as glp,
            tc.tile_pool(name="mp", bufs=8) as mp,
            tc.tile_pool(name="fl", bufs=4) as flp,
            tc.tile_pool(name="acc", bufs=3, space="PSUM") as accp,
            tc.tile_pool(name="tps", bufs=2, space="PSUM") as tpsp,
            tc.tile_pool(name="gps", bufs=2, space="PSUM") as gpsp,
        ):
            # constants
            w1_t = constp.tile([D, D], BF16_T, tag="w1")
            nc.sync.dma_start(out=w1_t[:], in_=w1[:, :])
            w2_t = constp.tile([D, D], BF16_T, tag="w2")
            nc.sync.dma_start(out=w2_t[:], in_=w2[:, :])
            wfc_t = constp.tile([D, DOUT], BF16_T, tag="wfc")
            nc.sync.dma_start(out=wfc_t[:], in_=wfc[:, :])
            brow_ts = []
            for r in range(4):
                bt = constp.tile([1, D], BF16_T, tag=f"brow{r}")
                nc.sync.dma_start(out=bt[:], in_=brows[r : r + 1, :])
                brow_ts.append(bt)
            dinv_t = constp.tile([128, n_tiles], FP32, tag="dinvp")
            nc.sync.dma_start(out=dinv_t[:], in_=dinvp[:, :])
            if has_bias:
                sqdeg_t = constp.tile([1, slice_n], BF16_T, tag="sqdegp")
                nc.sync.dma_start(out=sqdeg_t[:], in_=sqdegp[:, :])
            ident_t = constp.tile([128, 128], BF16_T, tag="ident")
            from concourse.masks import make_identity
            make_identity(nc, ident_t[:])
            ident8_t = constp.tile([128, 128], FP8, tag="ident8")
            make_identity(nc, ident8_t[:])

            # ---------------- Stage A ----------------
            # one bulk input load per half; matmuls slice it; grouped stores
            def ag(bounce_h, table_h):
                nc.gpsimd.collective_compute(
                    "AllGather", mybir.AluOpType.bypass,
                    ins=[bounce_h[:, :]], outs=[table_h[:, :]],
                    replica_groups=[list(range(N_CORES))],
                )

            # ---------------- aggregation layers ----------------
            idx_all = constp.tile([128, s_total], I16, tag="idxall")
            nc.sync.dma_start(out=idx_all[:], in_=idx1[:, :])
            _regs = {}

            def nidx_reg(v):
                if v not in _regs:
                    _regs[v] = nc.gpsimd.to_reg(v)
                return _regs[v]

            n_batches = len(meta["batches"])
            per_q = n_batches // 4

            def issue_batch(idx_dram, tables, bounces, spans, bi, tiles):
                bm = meta["batch_meta"][bi]
                s0, slen = spans[bi]
                it = idx_all[:, s0 : s0 + slen]

                # host-precomputed one-hot tiles, one per dst tile
                mts = []
                for ti, t in enumerate(tiles):
                    i0 = meta["inst_start"][(bi, t)]
                    ni = len(bm["tiles"][t])
                    m_t = mp.tile([128, maxi, 128], FP8, tag="m")
                    eng = nc.sync if ti < 3 else nc.scalar
                    eng.dma_start(
                        out=m_t[:, :ni, :],
                        in_=mhot[:, i0 * 128 : (i0 + ni) * 128]
                            .rearrange("p (i f) -> p i f", f=128),
                    )
                    mts.append(m_t)

                # local g rows for the batch's self-loop matmuls:
                # glt[p, j, :] = bounce_half[row0 + j*TILE_N + p, :]
                ntl = len(tiles)
                q = 0 if (tiles[-1] + 1) * TILE_N <= PR[0] else 1
                row0 = tiles[0] * TILE_N - (0 if q == 0 else PR[0])
                glt = glp.tile([128, ntl, DW], FP8, tag="gl")
                nc.sync.dma_start(
                    out=glt[:TILE_N, :, :],
                    in_=bounces[q][row0 : row0 + ntl * TILE_N, :]
                        .rearrange("(t p) d -> p t d", p=TILE_N),
                )

                # bank gathers, split to <= MAXC chunks per msg tile
                chunk_map = {}      # global col -> (msg tile, local col)
                off16 = 0
                for k, c0, nbk in bm["banks"]:
                    if nbk == 0:
                        continue
                    tab = tables[0] if k < 3 else tables[1]
                    r0 = k * BANK if k < 3 else 0
                    rows = BANK
                    for g0 in range(0, nbk, MAXC):
                        gl = min(MAXC, nbk - g0)
                        mt = msgp.tile([128, gl, DW], FP8, tag="msg")
                        nc.gpsimd.dma_gather(
                            out_ap=mt[:],
                            in_ap=tab[r0 : r0 + rows, :],
                            idxs_ap=it[:, off16 : off16 + gl * 8],
                            num_idxs=gl * CHUNK,
                            num_idxs_reg=nidx_reg(gl * CHUNK),
                            elem_size=DW,
                            single_packet=False,
                            queue_num=k % 4,
                        )
                        for j in range(gl):
                            chunk_map[c0 + g0 + j] = (mt, j)
                        off16 += gl * 8
                return chunk_map, mts, glt

            def agg_layer(idx_dram, tables, bounces, spans, brow_idx,
                          flush, post=None, lookahead=3):
                batches = meta["batches"]
                pending = []
                for bi, tiles in enumerate(batches):
                    pending.append(
                        (bi, tiles,
                         *issue_batch(idx_dram, tables, bounces,
                                      spans, bi, tiles))
                    )
                    if len(pending) > lookahead or bi == len(batches) - 1:
                        todo = pending if bi == len(batches) - 1 else [pending.pop(0)]
                        for job in todo:
                            process_batch(job, brow_idx, flush)
                            if post and job[0] in post:
                                post[job[0]]()
                        if bi == len(batches) - 1:
                            pending = []

            def process_batch(job, brow_idx, flush):
                bi, tiles, chunk_map, mts, glt = job
                bm = meta["batch_meta"][bi]
                for tl, t in enumerate(tiles):
                    acc = accp.tile([128, D], FP32, tag="acc")
                    # self-loop contribution: acc += I @ g_local[tile]
                    chunk_list = bm["tiles"][t]
                    nch_t = len(chunk_list)
                    nc.tensor.matmul(
                        out=acc[:, :],
                        lhsT=ident8_t[:TILE_N, :],
                        rhs=glt[:TILE_N, tl, :D],
                        start=True, stop=(nch_t == 0),
                    )
                    m_t = mts[tl]
                    for ci, (k, col) in enumerate(chunk_list):
                        mt, j = chunk_map[col]
                        nc.tensor.matmul(
                            out=acc[:, :],
                            lhsT=m_t[:, ci, :],
                            rhs=mt[:, j, :D],
                            start=False,
                            stop=(ci + 1 == nch_t),
                        )
                    flush(t, acc)

            def wmm(t, acc, w_t, bias_row):
                """SBUF copy + transpose acc, then (acc @ W) [+ sqdeg x b]"""
                ab = flp.tile([128, D], BF16_T, tag="ab")
                nc.vector.tensor_copy(out=ab[:TILE_N, :], in_=acc[:TILE_N, :])
                abp = tpsp.tile([128, TILE_N], BF16_T, tag="stp")
                nc.tensor.transpose(out=abp[:], in_=ab[:TILE_N, :],
                                    identity=ident_t[:TILE_N, :TILE_N])
                abt = flp.tile([128, 128], BF16_T, tag="stb")
                nc.vector.tensor_copy(out=abt[:, :TILE_N], in_=abp[:])
                hp = gpsp.tile([128, D], FP32, tag="hp")
                nc.tensor.matmul(out=hp[:, :], lhsT=abt[:],
                                 rhs=w_t[:], start=True, stop=not has_bias)
                if has_bias:
                    nc.tensor.matmul(
                        out=hp[:TILE_N, :],
                        lhsT=sqdeg_t[:, t * TILE_N : (t + 1) * TILE_N],
                        rhs=bias_row[:, :],
                        start=False, stop=True,
                    )
                return hp

            # Stage B flush: h1 = relu(dinv*(acc@W1)); table row = dinv*h1
            def flush_b(t, acc):
                dv = dinv_t[:TILE_N, t : t + 1]
                hp = wmm(t, acc, w1_t, brow_ts[0])
                h1 = flp.tile([128, D], BF16_T, tag="h1")
                nc.scalar.activation(out=h1[:TILE_N, :], in_=hp[:TILE_N, :],
                                     func=mybir.ActivationFunctionType.Relu,
                                     scale=dv)
                g2sb = flp.tile([128, DW], FP8, tag="g2sb")
                nc.scalar.activation(out=g2sb[:TILE_N, :D], in_=h1[:TILE_N, :],
                                     func=mybir.ActivationFunctionType.Copy,
                                     scale=dv)
                q = 0 if t < 75 else 1
                r0 = t * TILE_N - (0 if t < 75 else PR[0])
                nc.scalar.dma_start(
                    out=g2_bnc[q][r0 : r0 + TILE_N, :],
                    in_=g2sb[:TILE_N, :],
                )

            # Stage C flush: h2 = relu(dinv*(acc@W2)); logits; log_softmax
            def flush_c(t, acc):
                dv = dinv_t[:TILE_N, t : t + 1]
                hp = wmm(t, acc, w2_t, brow_ts[1])
                h2 = flp.tile([128, D], BF16_T, tag="h1")
                nc.scalar.activation(out=h2[:TILE_N, :], in_=hp[:TILE_N, :],
                                     func=mybir.ActivationFunctionType.Relu,
                                     scale=dv)
                htp = tpsp.tile([128, TILE_N], BF16_T, tag="stp")
                nc.tensor.transpose(out=htp[:], in_=h2[:TILE_N, :],
                                    identity=ident_t[:TILE_N, :TILE_N])
                htb = flp.tile([128, 128], BF16_T, tag="stb")
                nc.vector.tensor_copy(out=htb[:, :TILE_N], in_=htp[:])
                lgf = gpsp.tile([128, D], FP32, tag="hp")
                lg = lgf[:, :DOUT]
                nc.tensor.matmul(out=lg, lhsT=htb[:],
                                 rhs=wfc_t[:, :DOUT], start=True, stop=not has_bias)
                if has_bias:
                    nc.tensor.matmul(out=lgf[:TILE_N, :DOUT],
                                     lhsT=brow_ts[3][:, :TILE_N],
                                     rhs=brow_ts[2][:, :DOUT],
                                     start=False, stop=True)
                mx = flp.tile([128, 1], FP32, tag="mx")
                nc.vector.tensor_reduce(out=mx[:TILE_N, :], in_=lgf[:TILE_N, :DOUT],
                                        axis=mybir.AxisListType.X,
                                        op=mybir.AluOpType.max)
                negm = flp.tile([128, 1], FP32, tag="negm")
                nc.vector.tensor_scalar_mul(out=negm[:TILE_N, :],
                                            in0=mx[:TILE_N, :], scalar1=-1.0)
                esc = flp.tile([128, DOUT], BF16_T, tag="esc")
                ssum = flp.tile([128, 1], FP32, tag="ssum")
                nc.scalar.activation(out=esc[:TILE_N, :], in_=lgf[:TILE_N, :DOUT],
                                     func=mybir.ActivationFunctionType.Exp,
                                     bias=negm[:TILE_N, :],
                                     accum_out=ssum[:TILE_N, :])
                lns = flp.tile([128, 1], FP32, tag="lns")
                nc.scalar.activation(out=lns[:TILE_N, :], in_=ssum[:TILE_N, :],
                                     func=mybir.ActivationFunctionType.Ln)
                nmls = flp.tile([128, 1], FP32, tag="nmls")
                nc.vector.tensor_tensor(out=nmls[:TILE_N, :],
                                        in0=negm[:TILE_N, :],
                                        in1=lns[:TILE_N, :],
                                        op=mybir.AluOpType.subtract)
                ot = flp.tile([128, DOUT], FP32, tag="ot")
                nc.vector.tensor_tensor(out=ot[:TILE_N, :], in0=lgf[:TILE_N, :DOUT],
                                        in1=nmls[:TILE_N, :]
                                            .to_broadcast([TILE_N, DOUT]),
                                        op=mybir.AluOpType.add)
                nc.scalar.dma_start(
                    out=out[t * TILE_N : (t + 1) * TILE_N, :],
                    in_=ot[:TILE_N, :],
                )

            agg_layer(idx1, xgt, xgo, meta["spans1"], 0,
                      flush_b,
                      post={15: lambda: ag(g2_bnc[0], g2_tab[0]),
                            n_batches - 1: lambda: ag(g2_bnc[1], g2_tab[1])})
            agg_layer(idx1, g2_tab, g2_bnc, meta["spans2"], 1,
                      flush_c)

    nc.compile()
    return nc


# ---------------------------------------------------------------------------
# Public entry point
# ---------------------------------------------------------------------------

_CACHE = {}


def kernel(x, edge_index, W1, b1, W2, b2, Wfc, bfc):
    x = np.asarray(x, np.float32)
    per_core, meta, dinv, sqdeg = _preprocess(x, edge_index)
    n, slice_n = meta["n"], meta["slice_n"]

    # same static structure for both layers -> same spans
    idx_packs, mhot_packs = [], []
    spans = None
    fidx = np.arange(128, dtype=np.int16)
    for c in range(N_CORES):
        idx_mat, seg_mat = per_core[c]
        packed, spans = _pack_idx(idx_mat, meta)
        idx_packs.append(packed)
        # host-built one-hot matrices, partition-major [128, n_inst*TILE_N]
        S = seg_mat[meta["inst_cols"], :]               # [n_inst, 128]
        targ = meta["inst_off"][:, None, None] + fidx[None, None, :]
        mh = (S[:, :, None] == targ).astype(F8NP)       # [n_inst, 128, 128]
        mh = np.ascontiguousarray(mh.transpose(1, 0, 2)).reshape(128, -1)
        mhot_packs.append(mh)
    meta["spans1"] = spans
    meta["spans2"] = spans
    s_total = idx_packs[0].shape[1]

    has_bias = bool(
        np.any(np.asarray(b1)) or np.any(np.asarray(b2)) or np.any(np.asarray(bfc))
    )
    key = (n, meta["total_chunks"], s_total, has_bias)
    if key not in _CACHE:
        _CACHE[key] = _build(meta, s_total, has_bias)
    nc = _CACHE[key]

    # full dinv*x gather table in fp8, padded 256B rows, permuted row order
    xg8 = np.zeros((n, 256), F8NP)
    xg8[:, :D] = dinv[:, None] * x
    xgp = np.zeros_like(xg8)
    xgp[meta["perm"], :] = xg8
    P0 = (slice_n // 4) * 3
    xgt0 = np.ascontiguousarray(xgp[: N_CORES * P0])
    xgt1 = np.ascontiguousarray(xgp[N_CORES * P0 :])
    brows = np.zeros((4, D), np.float32)
    brows[0, :] = np.asarray(b1, np.float32)
    brows[1, :] = np.asarray(b2, np.float32)
    brows[2, : DOUT] = np.asarray(bfc, np.float32)
    brows[3, :] = 1.0
    brows = brows.astype(BF16)

    in_maps = []
    for c in range(N_CORES):
        s0 = c * slice_n
        im = {
            "xgt0": xgt0,
            "xgt1": xgt1,
            "xgo0": np.ascontiguousarray(xg8[s0 : s0 + P0]),
            "xgo1": np.ascontiguousarray(xg8[s0 + P0 : s0 + slice_n]),
            "w1": np.asarray(W1, np.float32).astype(BF16),
            "w2": np.asarray(W2, np.float32).astype(BF16),
            "wfc": np.asarray(Wfc, np.float32).astype(BF16),
            "brows": brows,
            "dinvp": _pack_dinv(dinv, s0, meta, np.float32),
            "idx1": idx_packs[c],
            "mhot": mhot_packs[c],
        }
        if has_bias:
            im["sqdegp"] = sqdeg[s0 : s0 + slice_n][None, :].astype(BF16)
        in_maps.append(im)

    global _last_in_maps
    _last_in_maps = in_maps
    last_exc = None
    for _attempt in range(3):
        try:
            res = bass_utils.run_bass_kernel_spmd(
                nc, in_maps, core_ids=list(range(N_CORES))
            )
            return np.concatenate(
                [res.results[c]["out"] for c in range(N_CORES)], axis=0
            )
        except Exception as e:  # transient device/tunnel errors: retry
            last_exc = e
    raise last_exc


_last_in_maps = None
